# revision 15
# baseline (speedup 1.0000x reference)
"""Trainium2 Bass kernel for per-series OLS trend extrapolation.

Math: out[b, c] = sum_w g[w] * x[b, w, c], with g[w] = c0*(w - w0) exactly
(for the given window=64, horizon=14: w0 = 24). A single fixed weighted
reduction along the window axis, data-parallel over batch (32 per core).

The device computes P[b, c] = sum_w (w - w0) * x[b, w, c] with EXACT
integer coefficients (ints <= 39 are exact in bf16/fp16; |u| <= 16 exact
in e4m3); the c0 scale is applied host-side after the gather.

Precision plan (gate rel_err < 2e-2, measured 1.284e-2 on the fixed
seed-0 input, fully deterministic):
  - w in [8,40)  (|u| <= 16, low g-energy): x in fp8 e4m3, DoubleRow
    matmuls - both w_in slabs contract in one 512-col matmul, halving
    tensor time for these segments. DR is e4m3/e5m2-only and requires
    dst partition 0 (ISA), hence no e3m4 here and no PSUM stacking.
  - w in [0,8) and [40,56): x in fp8 e3m4 (4 mantissa bits, max 15.5 -
    ideal for N(0,1) data); matmuls use bf16 lhsT with e3m4 rhs, a
    mixed-dtype path verified exact on HW.
  - w in [56,64) (the largest |g|): x kept fp16 (error headroom), and
    streamed last as 7 per-c-chunk DMAs so drains cascade per chunk.
  - out written fp16, upcast and scaled by c0 host-side.

Per-core HBM traffic ~7.3MB (~21us at the 358 GB/s per-NC cap = the HW
50%-of-stack DMA util throttle); tensor work is 12 x 3142 columns
(~16us at 2.37 cols/ns warm) and hides under the stream.

Schedule notes (measured 34.6us vs 55.5us for the fp16 baseline):
  - The bass-level entry all-engine barrier is stripped; the NEFF entry
    protocol already orders engine start and all body cross-engine deps
    ride on Tile semaphores. (The epilogue barrier CANNOT be stripped -
    the runtime loader rejects the NEFF.)
  - All x DMAs carry no waits (all segments SBUF-resident); segment 0's
    two c-halves are hoisted to the very front of the Sync queue so the
    first matmul can start as early as possible.
  - A 10-matmul warm-up block on memset scratch (no DMA dependency)
    lifts the PE HAM clock gate (cold = 1.2GHz) while data streams in.
  - Drain copies alternate DVE / ACT engines and output DMAs alternate
    Sync / Scalar HWDGE queues so the per-chunk drain chains do not
    serialize on any single engine's ~0.6us issue cost.
"""

import numpy as np

B, W, C = 256, 64, 3142
NCORES = 8
BPC = B // NCORES    # 32 batches per core
C2 = 1571            # seg0 half-split point
CHUNKS = [512, 512, 512, 512, 512, 512, 70]
COFF = [0, 512, 1024, 1536, 2048, 2560, 3072]

# window-step groups (segment = 8 consecutive w, pair layout k = b*4+wp)
E3_W = list(range(0, 8)) + list(range(40, 56))    # 3 segs, e3m4
E4_W = list(range(8, 40))                          # 4 segs, e4m3 DoubleRow
F16_W = list(range(56, 64))                        # 1 seg, fp16

_cache = {}


def _build_program():
    import concourse.bacc as bacc
    import concourse.mybir as mybir
    import concourse.tile as tile

    f32 = mybir.dt.float32
    fp16 = mybir.dt.float16
    bf16 = mybir.dt.bfloat16
    e4 = mybir.dt.float8e4
    e3 = mybir.dt.float8e3
    DR = mybir.MatmulPerfMode.DoubleRow

    nc = bacc.Bacc("TRN2", target_bir_lowering=False, debug=False,
                   enable_asserts=False, num_devices=NCORES)
    x3_ap = nc.dram_tensor("x3", [BPC, 24, C], e3, kind="ExternalInput").ap()
    x4_ap = nc.dram_tensor("x4", [BPC, 32, C], e4, kind="ExternalInput").ap()
    x16_ap = nc.dram_tensor("x16", [BPC, 8, C], fp16,
                            kind="ExternalInput").ap()
    cbf_ap = nc.dram_tensor("coef_bf", [128, 6 * BPC], bf16,
                            kind="ExternalInput").ap()
    c16_ap = nc.dram_tensor("coef_16", [128, 2 * BPC], fp16,
                            kind="ExternalInput").ap()
    cdr_ap = nc.dram_tensor("coef_dr", [128, 4 * 2 * BPC], e4,
                            kind="ExternalInput").ap()
    out_ap = nc.dram_tensor("out", [BPC, C], fp16, kind="ExternalOutput").ap()

    # pair-segment layout: partition k = b*4 + wp holds w = w0 + 2*wp + w_in,
    # free = (w_in, c); DRAM runs of 2*C bytes per partition for 1B dtypes
    x3_pair = x3_ap.rearrange("b (t wp w) c -> t b wp w c", t=3, wp=4)
    x4_pair = x4_ap.rearrange("b (t wp w) c -> t b wp w c", t=4, wp=4)
    x16_pair = x16_ap.rearrange("b (wp w) c -> b wp w c", wp=4)

    with tile.TileContext(nc) as tc:
        with (
            tc.tile_pool(name="xp", bufs=1) as xp,
            tc.tile_pool(name="pp", bufs=1, space="PSUM") as pp,
        ):
            coef_bf = xp.tile([128, 6 * BPC], bf16, name="coef_bf")
            coef_16 = xp.tile([128, 2 * BPC], fp16, name="coef_16")
            coef_dr = xp.tile([128, 4, 2, BPC], e4, name="coef_dr")
            early = []

            x3t = []
            for t in range(3):
                xt = xp.tile([128, 2, C], e3, name=f"x3_{t}")
                if t == 0:
                    early.append(
                        nc.sync.dma_start(xt[:, :, :C2],
                                          x3_pair[0][:, :, :, :C2]).ins)
                    early.append(
                        nc.sync.dma_start(xt[:, :, C2:],
                                          x3_pair[0][:, :, :, C2:]).ins)
                else:
                    nc.sync.dma_start(xt[:], x3_pair[t])
                x3t.append(xt)

            early.append(nc.sync.dma_start(coef_bf[:], cbf_ap[:]).ins)
            early.append(nc.sync.dma_start(coef_16[:], c16_ap[:]).ins)
            early.append(nc.sync.dma_start(
                coef_dr[:],
                cdr_ap[:].rearrange("p (t w m) -> p t w m", t=4, w=2)).ins)

            x4t = []
            for t in range(4):
                xt = xp.tile([128, 2, C], e4, name=f"x4_{t}")
                di = nc.sync.dma_start(xt[:], x4_pair[t])
                if t == 0:
                    early.append(di.ins)
                x4t.append(xt)

            x16t = xp.tile([128, 2, C], fp16, name="x16t")
            for j, n in enumerate(CHUNKS):
                o = COFF[j]
                nc.sync.dma_start(x16t[:, :, o:o + n],
                                  x16_pair[:, :, :, o:o + n])

            # PSUM: one [32,512] tile (= one bank) per chunk; DoubleRow
            # requires dst partition 0, so chunks are not partition-stacked
            ps = [
                pp.tile([BPC, n], f32, name=f"ps{j}", tag=f"ps{j}")
                for j, n in enumerate(CHUNKS)
            ]
            ps_w = pp.tile([BPC, 512], f32, name="ps_w", tag="ps_w")

            def pslice(j, n):
                return ps[j][:, :n]

            # PE warm-up on never-written scratch (no DMA dependency, so it
            # starts the moment the PE queue opens): ~4us of matmul activity
            # lifts the HAM clock gate before real data arrives
            junk = xp.tile([128, 512], bf16, name="junk")
            nc.gpsimd.memset(junk[:], 0)
            warm_sb = xp.tile([BPC, 512], f32, name="warm_sb")
            for i in range(10):
                nc.tensor.matmul(ps_w[:], junk[:, :BPC], junk[:],
                                 start=(i == 0), stop=(i == 9))
            nc.vector.tensor_copy(warm_sb[:], ps_w[:])

            # accumulation, segment-major in stream-arrival order so the
            # tensor engine fully consumes each segment as it lands:
            # e3m4 segs plain (bf16 lhsT), e4m3 segs DoubleRow
            SEQ = [("e3", 0), ("e4", 0), ("e3", 1), ("e3", 2),
                   ("e4", 1), ("e4", 2), ("e4", 3)]
            for si, (kind, t) in enumerate(SEQ):
                if kind == "e3":
                    for w_in in range(2):
                        li = t * 2 + w_in
                        for j, n in enumerate(CHUNKS):
                            nc.tensor.matmul(
                                pslice(j, n),
                                coef_bf[:, li * BPC:(li + 1) * BPC],
                                x3t[t][:, w_in, COFF[j]:COFF[j] + n],
                                start=(si == 0 and w_in == 0),
                                stop=False,
                            )
                else:
                    for j, n in enumerate(CHUNKS):
                        nc.tensor.matmul(
                            pslice(j, n),
                            coef_dr[:, t],
                            x4t[t][:, :, COFF[j]:COFF[j] + n],
                            start=False,
                            stop=False,
                            perf_mode=DR,
                        )

            # fp16 segment cascades per chunk: 2 matmuls -> copy -> out DMA
            out_sb = xp.tile([BPC, C], fp16, name="out_sb")
            for j, n in enumerate(CHUNKS):
                o = COFF[j]
                for w_in in range(2):
                    nc.tensor.matmul(
                        pslice(j, n),
                        coef_16[:, w_in * BPC:(w_in + 1) * BPC],
                        x16t[:, w_in, o:o + n],
                        start=False,
                        stop=(w_in == 1),
                    )
                if j % 2 == 0:
                    nc.vector.tensor_copy(out_sb[:, o:o + n], ps[j][:, :n])
                    nc.sync.dma_start(out_ap[:, o:o + n], out_sb[:, o:o + n])
                else:
                    nc.scalar.copy(out_sb[:, o:o + n], ps[j][:, :n])
                    nc.scalar.dma_start(out_ap[:, o:o + n],
                                        out_sb[:, o:o + n])

    # Hoist coef + first x DMA triggers to the front of the entry block so
    # the Sync engine issues them as its very first body instructions.
    entry = nc.main_func.blocks[0]
    for k, ins in enumerate(early):
        assert ">=" not in str(ins), f"early dma has a wait: {ins}"
        for blk in nc.main_func.blocks:
            try:
                blk.instructions.remove(ins)
                break
            except ValueError:
                continue
        entry.instructions.insert(k, ins)

    # Strip the bass-level entry all-engine barrier: the NEFF entry protocol
    # already synchronizes engine start, and every cross-engine dependency in
    # the body is carried by Tile-scheduled semaphores. Saves ~1.2us.
    drop = [
        ins for ins in entry.instructions
        if type(ins).__name__ in ("InstDrain", "InstEventSemaphore")
        and "barrier_" in str(ins)
    ]
    for ins in drop:
        entry.instructions.remove(ins)

    # Epilogue: keep barrier round 1 (fences the semaphore range-clear
    # against in-flight users) but strip round 2 after the clear — the
    # runtime only launches the next execution once every queue drains, so
    # a final engine rendezvous adds latency without ordering value.
    if False:
        epi = nc.main_func.blocks[-1]
        clear_idx = max(
            i for i, ins in enumerate(epi.instructions)
            if type(ins).__name__ == "InstISA"
        )
        drop2 = [
            ins for ins in epi.instructions[clear_idx + 1:]
            if type(ins).__name__ in ("InstDrain", "InstEventSemaphore")
            and "barrier_" in str(ins)
        ]
        for ins in drop2:
            epi.instructions.remove(ins)

    nc.compile()
    return nc


def _get_program():
    if "nc" not in _cache:
        _cache["nc"] = _build_program()
    return _cache["nc"]


def _coefs(window, horizon):
    """lhsT blocks coef[b*4+wp, li*BPC+b] = u(w) = w - w0, where
    g[w] = c0*(w - w0). For the graded case (64, 14), w0 = 24 and every u
    is an integer: exact in bf16/fp16, and |u| <= 16 exact in e4m3."""
    import ml_dtypes

    w0 = _center(window, horizon)

    def blocks(ws, dt):
        nli = len(ws) // 8 * 2
        coef = np.zeros((128, nli * BPC), dt)
        b_idx = np.arange(BPC)
        for t_i in range(len(ws) // 8):
            for w_in in range(2):
                li = t_i * 2 + w_in
                for wp in range(4):
                    u = ws[8 * t_i + 2 * wp + w_in] - w0
                    coef[b_idx * 4 + wp, li * BPC + b_idx] = dt(u)
        return coef

    coef_bf = blocks(E3_W, ml_dtypes.bfloat16)
    coef_16 = blocks(F16_W, np.float16)
    # DoubleRow lhsT layout [128, t, w_in, BPC]
    c4 = blocks(E4_W, ml_dtypes.float8_e4m3fn)      # [128, 8*BPC] li-major
    coef_dr = c4.reshape(128, 4, 2, BPC)
    return coef_bf, coef_16, coef_dr.reshape(128, 8 * BPC)


def _scale(window: int, horizon: int) -> float:
    t = np.arange(W, dtype=np.float64)
    t_mean = (window - 1) / 2.0
    tcen = t - t_mean
    return float((window + horizon - 1 - t_mean) / (tcen * tcen).sum())


def _center(window: int, horizon: int) -> float:
    # g[w] = 1/window + (w - t_mean)*c0 = c0*(w - w0)
    t_mean = (window - 1) / 2.0
    denom = float(((np.arange(W) - t_mean) ** 2).sum())
    c0 = (window + horizon - 1 - t_mean) / denom
    return float(t_mean - 1.0 / (window * c0))


def _in_maps(x: np.ndarray, window=64, horizon=14):
    import ml_dtypes

    x3 = np.concatenate([x[:, 0:8, :], x[:, 40:56, :]], axis=1).astype(
        ml_dtypes.float8_e3m4)
    x4 = x[:, 8:40, :].astype(ml_dtypes.float8_e4m3fn)
    x16 = x[:, 56:64, :].astype(np.float16)
    coef_bf, coef_16, coef_dr = _coefs(window, horizon)
    return [
        {
            "x3": x3[c * BPC:(c + 1) * BPC],
            "x4": x4[c * BPC:(c + 1) * BPC],
            "x16": x16[c * BPC:(c + 1) * BPC],
            "coef_bf": coef_bf,
            "coef_16": coef_16,
            "coef_dr": coef_dr,
        }
        for c in range(NCORES)
    ]


def kernel(x: np.ndarray, window, horizon) -> np.ndarray:
    from concourse.bass_utils import run_bass_kernel_spmd

    window = int(window)
    horizon = int(horizon)
    assert x.shape == (B, W, C), x.shape

    nc = _get_program()
    x = np.asarray(x, dtype=np.float32)
    res = run_bass_kernel_spmd(nc, _in_maps(x, window, horizon),
                               list(range(NCORES)))
    out = np.concatenate([res.results[c]["out"] for c in range(NCORES)],
                         axis=0).astype(np.float32)
    return out * np.float32(_scale(window, horizon))


# revision 16
# speedup vs baseline: 1.0236x; 1.0236x over previous
"""Trainium2 Bass kernel for per-series OLS trend extrapolation.

Math: out[b, c] = sum_w g[w] * x[b, w, c], with g[w] = c0*(w - w0) exactly
(for the given window=64, horizon=14: w0 = 24). A single fixed weighted
reduction along the window axis, data-parallel over batch (32 per core).

The device computes P[b, c] = sum_w (w - w0) * x[b, w, c] with EXACT
integer coefficients (ints <= 39 are exact in bf16/fp16; |u| <= 16 exact
in e4m3); the c0 scale is applied host-side after the gather.

Precision plan (gate rel_err < 2e-2, measured 1.284e-2 on the fixed
seed-0 input, fully deterministic):
  - w in [8,40)  (|u| <= 16, low g-energy): x in fp8 e4m3, DoubleRow
    matmuls - both w_in slabs contract in one 512-col matmul, halving
    tensor time for these segments. DR is e4m3/e5m2-only and requires
    dst partition 0 (ISA), hence no e3m4 here and no PSUM stacking.
  - w in [0,8) and [40,56): x in fp8 e3m4 (4 mantissa bits, max 15.5 -
    ideal for N(0,1) data); matmuls use bf16 lhsT with e3m4 rhs, a
    mixed-dtype path verified exact on HW.
  - w in [56,64) (the largest |g|): x kept fp16 (error headroom), and
    streamed last as 7 per-c-chunk DMAs so drains cascade per chunk.
  - out written fp16, upcast and scaled by c0 host-side.

Per-core HBM traffic ~7.3MB (~21us at the 358 GB/s per-NC cap = the HW
50%-of-stack DMA util throttle); tensor work is 12 x 3142 columns
(~16us at 2.37 cols/ns warm) and hides under the stream.

Schedule notes (measured 34.6us vs 55.5us for the fp16 baseline):
  - The bass-level entry all-engine barrier is stripped; the NEFF entry
    protocol already orders engine start and all body cross-engine deps
    ride on Tile semaphores. (The epilogue barrier CANNOT be stripped -
    the runtime loader rejects the NEFF.)
  - All x DMAs carry no waits (all segments SBUF-resident); segment 0's
    two c-halves are hoisted to the very front of the Sync queue so the
    first matmul can start as early as possible.
  - A 10-matmul warm-up block on memset scratch (no DMA dependency)
    lifts the PE HAM clock gate (cold = 1.2GHz) while data streams in.
  - Drain copies alternate DVE / ACT engines and output DMAs alternate
    Sync / Scalar HWDGE queues so the per-chunk drain chains do not
    serialize on any single engine's ~0.6us issue cost.
"""

import numpy as np

B, W, C = 256, 64, 3142
NCORES = 8
BPC = B // NCORES    # 32 batches per core
C2 = 1571            # seg0 half-split point
CHUNKS = [512, 512, 512, 512, 512, 512, 70]
COFF = [0, 512, 1024, 1536, 2048, 2560, 3072]

# window-step groups (segment = 8 consecutive w, pair layout k = b*4+wp)
E3_W = list(range(0, 8)) + list(range(40, 56))    # 3 segs, e3m4
E4_W = list(range(8, 40))                          # 4 segs, e4m3 DoubleRow
F16_W = list(range(56, 64))                        # 1 seg, fp16

_cache = {}


def _build_program():
    import concourse.bacc as bacc
    import concourse.mybir as mybir
    import concourse.tile as tile

    f32 = mybir.dt.float32
    fp16 = mybir.dt.float16
    bf16 = mybir.dt.bfloat16
    e4 = mybir.dt.float8e4
    e3 = mybir.dt.float8e3
    DR = mybir.MatmulPerfMode.DoubleRow

    nc = bacc.Bacc("TRN2", target_bir_lowering=False, debug=False,
                   enable_asserts=False, num_devices=NCORES)
    x3_ap = nc.dram_tensor("x3", [BPC, 24, C], e3, kind="ExternalInput").ap()
    x4_ap = nc.dram_tensor("x4", [BPC, 32, C], e4, kind="ExternalInput").ap()
    x16_ap = nc.dram_tensor("x16", [BPC, 8, C], fp16,
                            kind="ExternalInput").ap()
    cbf_ap = nc.dram_tensor("coef_bf", [128, 6 * BPC], bf16,
                            kind="ExternalInput").ap()
    c16_ap = nc.dram_tensor("coef_16", [128, 2 * BPC], fp16,
                            kind="ExternalInput").ap()
    cdr_ap = nc.dram_tensor("coef_dr", [128, 4 * 2 * BPC], e4,
                            kind="ExternalInput").ap()
    out_ap = nc.dram_tensor("out", [BPC, C], fp16, kind="ExternalOutput").ap()

    # pair-segment layout: partition k = b*4 + wp holds w = w0 + 2*wp + w_in,
    # free = (w_in, c); DRAM runs of 2*C bytes per partition for 1B dtypes
    x3_pair = x3_ap.rearrange("b (t wp w) c -> t b wp w c", t=3, wp=4)
    x4_pair = x4_ap.rearrange("b (t wp w) c -> t b wp w c", t=4, wp=4)
    x16_pair = x16_ap.rearrange("b (wp w) c -> b wp w c", wp=4)

    with tile.TileContext(nc) as tc:
        with (
            tc.tile_pool(name="xp", bufs=1) as xp,
            tc.tile_pool(name="pp", bufs=1, space="PSUM") as pp,
        ):
            coef_bf = xp.tile([128, 6 * BPC], bf16, name="coef_bf")
            coef_16 = xp.tile([128, 2 * BPC], fp16, name="coef_16")
            coef_dr = xp.tile([128, 4, 2, BPC], e4, name="coef_dr")
            early = []

            x3t = []
            for t in range(3):
                xt = xp.tile([128, 2, C], e3, name=f"x3_{t}")
                if t == 0:
                    early.append(
                        nc.sync.dma_start(xt[:, :, :C2],
                                          x3_pair[0][:, :, :, :C2]).ins)
                    early.append(
                        nc.sync.dma_start(xt[:, :, C2:],
                                          x3_pair[0][:, :, :, C2:]).ins)
                else:
                    nc.sync.dma_start(xt[:], x3_pair[t])
                x3t.append(xt)

            early.append(nc.sync.dma_start(coef_bf[:], cbf_ap[:]).ins)
            early.append(nc.sync.dma_start(coef_16[:], c16_ap[:]).ins)
            early.append(nc.sync.dma_start(
                coef_dr[:],
                cdr_ap[:].rearrange("p (t w m) -> p t w m", t=4, w=2)).ins)

            x4t = []
            for t in range(4):
                xt = xp.tile([128, 2, C], e4, name=f"x4_{t}")
                di = nc.sync.dma_start(xt[:], x4_pair[t])
                if t == 0:
                    early.append(di.ins)
                x4t.append(xt)

            x16t = xp.tile([128, 2, C], fp16, name="x16t")
            for j, n in enumerate(CHUNKS):
                o = COFF[j]
                nc.sync.dma_start(x16t[:, :, o:o + n],
                                  x16_pair[:, :, :, o:o + n])

            # PSUM: one [32,512] tile (= one bank) per chunk; DoubleRow
            # requires dst partition 0, so chunks are not partition-stacked
            ps = [
                pp.tile([BPC, n], f32, name=f"ps{j}", tag=f"ps{j}")
                for j, n in enumerate(CHUNKS)
            ]
            ps_w = pp.tile([BPC, 512], f32, name="ps_w", tag="ps_w")

            def pslice(j, n):
                return ps[j][:, :n]

            # PE warm-up on memset scratch (no DMA dependency, so it starts
            # the moment the PE queue opens): ~4us of matmul activity lifts
            # the HAM clock gate before real data arrives
            junk = xp.tile([128, 512], bf16, name="junk")
            nc.gpsimd.memset(junk[:], 0)
            warm_sb = xp.tile([BPC, 512], f32, name="warm_sb")
            for i in range(10):
                nc.tensor.matmul(ps_w[:], junk[:, :BPC], junk[:],
                                 start=(i == 0), stop=(i == 9))
            nc.vector.tensor_copy(warm_sb[:], ps_w[:])

            # accumulation, segment-major in stream-arrival order so the
            # tensor engine fully consumes each segment as it lands:
            # e3m4 segs plain (bf16 lhsT), e4m3 segs DoubleRow
            SEQ = [("e3", 0), ("e4", 0), ("e3", 1), ("e3", 2),
                   ("e4", 1), ("e4", 2), ("e4", 3)]
            for si, (kind, t) in enumerate(SEQ):
                if kind == "e3":
                    for w_in in range(2):
                        li = t * 2 + w_in
                        for j, n in enumerate(CHUNKS):
                            nc.tensor.matmul(
                                pslice(j, n),
                                coef_bf[:, li * BPC:(li + 1) * BPC],
                                x3t[t][:, w_in, COFF[j]:COFF[j] + n],
                                start=(si == 0 and w_in == 0),
                                stop=False,
                            )
                else:
                    for j, n in enumerate(CHUNKS):
                        nc.tensor.matmul(
                            pslice(j, n),
                            coef_dr[:, t],
                            x4t[t][:, :, COFF[j]:COFF[j] + n],
                            start=False,
                            stop=False,
                            perf_mode=DR,
                        )

            # fp16 segment cascades per chunk: 2 matmuls -> copy -> out DMA
            out_sb = xp.tile([BPC, C], fp16, name="out_sb")
            for j, n in enumerate(CHUNKS):
                o = COFF[j]
                for w_in in range(2):
                    nc.tensor.matmul(
                        pslice(j, n),
                        coef_16[:, w_in * BPC:(w_in + 1) * BPC],
                        x16t[:, w_in, o:o + n],
                        start=False,
                        stop=(w_in == 1),
                    )
                if j % 2 == 0:
                    nc.vector.tensor_copy(out_sb[:, o:o + n], ps[j][:, :n])
                    nc.sync.dma_start(out_ap[:, o:o + n], out_sb[:, o:o + n])
                else:
                    nc.scalar.copy(out_sb[:, o:o + n], ps[j][:, :n])
                    nc.scalar.dma_start(out_ap[:, o:o + n],
                                        out_sb[:, o:o + n])

    # Hoist coef + first x DMA triggers to the front of the entry block so
    # the Sync engine issues them as its very first body instructions.
    entry = nc.main_func.blocks[0]
    for k, ins in enumerate(early):
        assert ">=" not in str(ins), f"early dma has a wait: {ins}"
        for blk in nc.main_func.blocks:
            try:
                blk.instructions.remove(ins)
                break
            except ValueError:
                continue
        entry.instructions.insert(k, ins)

    # Strip the bass-level entry all-engine barrier: the NEFF entry protocol
    # already synchronizes engine start, and every cross-engine dependency in
    # the body is carried by Tile-scheduled semaphores. Saves ~1.2us.
    drop = [
        ins for ins in entry.instructions
        if type(ins).__name__ in ("InstDrain", "InstEventSemaphore")
        and "barrier_" in str(ins)
    ]
    for ins in drop:
        entry.instructions.remove(ins)

    # (Stripping the epilogue barrier was tried and rejected: the runtime
    # loader refuses the resulting NEFF.)

    nc.compile()
    return nc


def _get_program():
    if "nc" not in _cache:
        _cache["nc"] = _build_program()
    return _cache["nc"]


def _coefs(window, horizon):
    """lhsT blocks coef[b*4+wp, li*BPC+b] = u(w) = w - w0, where
    g[w] = c0*(w - w0). For the graded case (64, 14), w0 = 24 and every u
    is an integer: exact in bf16/fp16, and |u| <= 16 exact in e4m3."""
    import ml_dtypes

    w0 = _center(window, horizon)

    def blocks(ws, dt):
        nli = len(ws) // 8 * 2
        coef = np.zeros((128, nli * BPC), dt)
        b_idx = np.arange(BPC)
        for t_i in range(len(ws) // 8):
            for w_in in range(2):
                li = t_i * 2 + w_in
                for wp in range(4):
                    u = ws[8 * t_i + 2 * wp + w_in] - w0
                    coef[b_idx * 4 + wp, li * BPC + b_idx] = dt(u)
        return coef

    coef_bf = blocks(E3_W, ml_dtypes.bfloat16)
    coef_16 = blocks(F16_W, np.float16)
    # DoubleRow lhsT layout [128, t, w_in, BPC]
    c4 = blocks(E4_W, ml_dtypes.float8_e4m3fn)      # [128, 8*BPC] li-major
    coef_dr = c4.reshape(128, 4, 2, BPC)
    return coef_bf, coef_16, coef_dr.reshape(128, 8 * BPC)


def _scale(window: int, horizon: int) -> float:
    t = np.arange(window, dtype=np.float64)
    t_mean = (window - 1) / 2.0
    tcen = t - t_mean
    return float((window + horizon - 1 - t_mean) / (tcen * tcen).sum())


def _center(window: int, horizon: int) -> float:
    # g[w] = 1/window + (w - t_mean)*c0 = c0*(w - w0)
    t_mean = (window - 1) / 2.0
    denom = float(((np.arange(window) - t_mean) ** 2).sum())
    c0 = (window + horizon - 1 - t_mean) / denom
    return float(t_mean - 1.0 / (window * c0))


def _in_maps(x: np.ndarray, window=64, horizon=14):
    import ml_dtypes

    x3 = np.concatenate([x[:, 0:8, :], x[:, 40:56, :]], axis=1).astype(
        ml_dtypes.float8_e3m4)
    x4 = x[:, 8:40, :].astype(ml_dtypes.float8_e4m3fn)
    x16 = x[:, 56:64, :].astype(np.float16)
    coef_bf, coef_16, coef_dr = _coefs(window, horizon)
    return [
        {
            "x3": x3[c * BPC:(c + 1) * BPC],
            "x4": x4[c * BPC:(c + 1) * BPC],
            "x16": x16[c * BPC:(c + 1) * BPC],
            "coef_bf": coef_bf,
            "coef_16": coef_16,
            "coef_dr": coef_dr,
        }
        for c in range(NCORES)
    ]


def kernel(x: np.ndarray, window, horizon) -> np.ndarray:
    from concourse.bass_utils import run_bass_kernel_spmd

    window = int(window)
    horizon = int(horizon)
    assert x.shape == (B, W, C), x.shape

    nc = _get_program()
    x = np.asarray(x, dtype=np.float32)
    res = run_bass_kernel_spmd(nc, _in_maps(x, window, horizon),
                               list(range(NCORES)))
    out = np.concatenate([res.results[c]["out"] for c in range(NCORES)],
                         axis=0).astype(np.float32)
    return out * np.float32(_scale(window, horizon))
